# revision 12
# baseline (speedup 1.0000x reference)
"""Bayesian linear layer (reparameterized sample + KL) on 8 Trainium2 NeuronCores.

Reference computation (all fp32):
    weight = weight_mu + exp(weight_sigma) * eps_w          # [OUT, IN]
    bias   = bias_mu   + exp(bias_sigma)   * eps_b          # [OUT]
    out    = x @ weight.T + bias                            # [B, OUT]
    kl     = -0.5 * sum(1 + 2*ws - wm^2 - exp(2*ws))        # over weight
             -0.5 * sum(1 + 2*bs - bm^2 - exp(2*bs))        # over bias

Sharding: 2x4 mesh.  Batch is split in 2 (B_S=4096 rows/core), out_features in 4
(O_S=1024 cols/core).  Core c handles b_shard=c//4, o_shard=c%4.  This minimizes
per-core HBM traffic (x shard 67MB + weight params 50MB + out 17MB = 134MB) and
keeps the kernel PE-bound (~437us of fp32 matmul at 1 col/cycle with float32r).

Host-side layout work (part of the sharding strategy): x and the weight params
are passed to each core pre-transposed ([IN, *] layouts) so the contraction dim
lands on SBUF partitions with no on-device transposes.

Device kernel per core:
  phase 1: stream w-param tiles, build W^T[IN, O_S] in SBUF
           (W = mu + exp(sigma)*eps) and accumulate the KL partial sums.
  phase 2: stream x^T tiles [128, 32, 128]; for each 128-row batch tile
           accumulate out[128, O_S] over 32 k-tiles into PSUM (float32r
           matmuls), add bias, store.
  tail:    fold the KL partials into one scalar.
"""

import os
import sys

import numpy as np

try:
    import concourse.bass as bass  # noqa: F401
except ImportError:  # pragma: no cover
    sys.path.insert(0, "/opt/trn_rl_repo")

import concourse.bass as bass
import concourse.tile as tile
from concourse import bacc, bass_isa, mybir
from concourse.bass_utils import run_bass_kernel_spmd

P = 128
B_FULL, IN_FULL, OUT_FULL = 8192, 4096, 4096
B_SHARDS, O_SHARDS = 2, 4
N_CORES = 8

F32 = mybir.dt.float32
MM_DT = mybir.dt.float32r  # PE fast-fp32 mode: 1 cycle/row at N>=256

AF = mybir.ActivationFunctionType
OP = mybir.AluOpType
AX = mybir.AxisListType


def build_bayes_kernel(nc, IN, B_S, O_S, mm_dt=MM_DT):
    """Emit the per-core SPMD program. Returns nothing; tensors are declared on nc."""
    KT = IN // P        # k tiles
    BT = B_S // P       # batch tiles
    NFREE = 512         # matmul moving free dim (1 psum bank of fp32)
    OC = O_S // NFREE   # out chunks per batch tile

    xt = nc.dram_tensor("xt", [IN, B_S], mm_dt, kind="ExternalInput").ap()
    wmu = nc.dram_tensor("wmu", [IN, O_S], F32, kind="ExternalInput").ap()
    wsig = nc.dram_tensor("wsig", [IN, O_S], F32, kind="ExternalInput").ap()
    weps = nc.dram_tensor("weps", [IN, O_S], F32, kind="ExternalInput").ap()
    bmu = nc.dram_tensor("bmu", [1, O_S], F32, kind="ExternalInput").ap()
    bsig = nc.dram_tensor("bsig", [1, O_S], F32, kind="ExternalInput").ap()
    beps = nc.dram_tensor("beps", [1, O_S], F32, kind="ExternalInput").ap()
    out = nc.dram_tensor("out", [B_S, O_S], F32, kind="ExternalOutput").ap()
    kl = nc.dram_tensor("kl", [1, 1], F32, kind="ExternalOutput").ap()

    xt_r = xt.rearrange("(kt p) b -> p kt b", p=P)
    wmu_r = wmu.rearrange("(kt p) o -> p kt o", p=P)
    wsig_r = wsig.rearrange("(kt p) o -> p kt o", p=P)
    weps_r = weps.rearrange("(kt p) o -> p kt o", p=P)
    out_r = out.rearrange("(bt p) o -> p bt o", p=P)

    with tile.TileContext(nc) as tc:
        with (
            tc.tile_pool(name="wpool", bufs=1) as wpool,
            tc.tile_pool(name="gen", bufs=2) as gen,
            tc.tile_pool(name="xin", bufs=2) as xin,
            tc.tile_pool(name="ost", bufs=2) as ost,
            tc.tile_pool(name="misc", bufs=1) as misc,
            tc.tile_pool(name="psum", bufs=4, space="PSUM") as psum,
        ):
            # Persistent state
            w_tiles = [
                wpool.tile([P, O_S], mm_dt, tag=f"w{kt}", name=f"w{kt}")
                for kt in range(KT)
            ]
            n_cols = KT * OC
            ssig = misc.tile([P, n_cols], F32, tag="ssig")   # per-(ktile,ohalf) sum(sigma)
            smu2 = misc.tile([P, n_cols], F32, tag="smu2")   # sum(mu^2)
            sv2 = misc.tile([P, n_cols], F32, tag="sv2")     # sum(exp(2 sigma))
            b_bc = misc.tile([P, O_S], F32, tag="bbc")       # bias broadcast to all partitions
            klb = misc.tile([1, 4], F32, tag="klb")          # bias kl cells: ssig, smu2, sv2, tmp

            # Prefetch the first two x tiles so phase 2 is not queued behind the
            # whole 50MB weight-param stream on the DMA FIFO.
            xpre = []
            for bt in range(min(2, BT)):
                xs = xin.tile([P, KT, P], mm_dt, tag="xs", name=f"xpre{bt}")
                nc.sync.dma_start(xs, xt_r[:, :, bt * P : (bt + 1) * P])
                xpre.append(xs)

            # ---- Phase 1: W = mu + exp(sigma)*eps, and KL partial sums ----
            for kt in range(KT):
                for oh in range(OC):
                    sl = slice(oh * NFREE, (oh + 1) * NFREE)
                    col = kt * OC + oh
                    mu = gen.tile([P, NFREE], F32, tag="mu")
                    nc.sync.dma_start(mu, wmu_r[:, kt, sl])
                    sg = gen.tile([P, NFREE], F32, tag="sig")
                    nc.sync.dma_start(sg, wsig_r[:, kt, sl])
                    ep = gen.tile([P, NFREE], F32, tag="eps")
                    nc.sync.dma_start(ep, weps_r[:, kt, sl])

                    # W built in fp32 scratch; the final add writes the f32r
                    # matmul operand tile (single rounding to f32r).
                    wtmp = gen.tile([P, NFREE], F32, tag="wtmp")
                    nc.scalar.activation(wtmp, sg, AF.Exp)
                    nc.vector.tensor_tensor(wtmp, wtmp, ep, OP.mult)
                    nc.vector.tensor_tensor(w_tiles[kt][:, sl], wtmp, mu, OP.add)

                    scr = gen.tile([P, NFREE], F32, tag="scr")
                    nc.scalar.activation(
                        scr, sg, AF.Exp, scale=2.0, accum_out=sv2[:, col : col + 1]
                    )
                    scr2 = gen.tile([P, NFREE], F32, tag="scr")
                    nc.scalar.activation(
                        scr2, mu, AF.Square, accum_out=smu2[:, col : col + 1]
                    )
                    nc.vector.tensor_reduce(
                        ssig[:, col : col + 1], sg, AX.X, OP.add
                    )

            # ---- Bias: value + KL pieces ----
            # all SBUF operands of an engine op must share the start partition,
            # so every bias tensor lives at partition 0 of its own tile; dead
            # input tiles double as ACT dump targets for the accum ops.
            bs = misc.tile([1, O_S], F32, tag="bs")
            bm = misc.tile([1, O_S], F32, tag="bm")
            be = misc.tile([1, O_S], F32, tag="be")
            nc.sync.dma_start(bs, bsig)
            nc.sync.dma_start(bm, bmu)
            nc.sync.dma_start(be, beps)
            bv = b_bc[0:1, :]
            nc.vector.tensor_reduce(klb[:, 0:1], bs, AX.X, OP.add)
            nc.scalar.activation(bv, bs, AF.Exp)
            nc.vector.tensor_tensor(bv, bv, be, OP.mult)      # be dead after this
            nc.vector.tensor_tensor(bv, bv, bm, OP.add)
            nc.scalar.activation(
                be, bs, AF.Exp, scale=2.0, accum_out=klb[:, 2:3]
            )
            nc.scalar.activation(bs, bm, AF.Square, accum_out=klb[:, 1:2])
            nc.gpsimd.partition_broadcast(b_bc, bv)

            # ---- Phase 2: out[bt] = x[bt] @ W^T + bias ----
            for bt in range(BT):
                if bt < len(xpre):
                    xs = xpre[bt]
                else:
                    xs = xin.tile([P, KT, P], mm_dt, tag="xs")
                    nc.sync.dma_start(xs, xt_r[:, :, bt * P : (bt + 1) * P])
                for oc in range(OC):
                    sl = slice(oc * NFREE, (oc + 1) * NFREE)
                    ps = psum.tile([P, NFREE], F32, tag="ps")
                    for kt in range(KT):
                        nc.tensor.matmul(
                            ps,
                            xs[:, kt, :],
                            w_tiles[kt][:, sl],
                            start=(kt == 0),
                            stop=(kt == KT - 1),
                        )
                    osb = ost.tile([P, NFREE], F32, tag="osb")
                    nc.vector.tensor_tensor(osb, ps, b_bc[:, sl], OP.add)
                    nc.sync.dma_start(out_r[:, bt, sl], osb)

            # ---- KL tail ----
            rs = misc.tile([P, 1], F32, tag="rs")
            rm = misc.tile([P, 1], F32, tag="rm")
            rv = misc.tile([P, 1], F32, tag="rv")
            nc.vector.tensor_reduce(rs, ssig, AX.X, OP.add)
            nc.vector.tensor_reduce(rm, smu2, AX.X, OP.add)
            nc.vector.tensor_reduce(rv, sv2, AX.X, OP.add)
            tcol = misc.tile([P, 1], F32, tag="tcol")
            # tcol = 2*rs - rm - rv
            nc.vector.scalar_tensor_tensor(tcol, rs, 2.0, rm, OP.mult, OP.subtract)
            nc.vector.tensor_tensor(tcol, tcol, rv, OP.subtract)
            # fold bias terms into partition 0: klb[3] = 2*ssig_b - smu2_b - sv2_b
            nc.vector.scalar_tensor_tensor(
                klb[:, 3:4], klb[:, 0:1], 2.0, klb[:, 1:2], OP.mult, OP.subtract
            )
            nc.vector.tensor_tensor(klb[:, 3:4], klb[:, 3:4], klb[:, 2:3], OP.subtract)
            nc.vector.tensor_tensor(tcol[0:1, :], tcol[0:1, :], klb[:, 3:4], OP.add)
            tall = misc.tile([P, 1], F32, tag="tall")
            nc.gpsimd.partition_all_reduce(tall, tcol, P, bass_isa.ReduceOp.add)
            # kl = -0.5 * (count + sum(2s - m^2 - v^2))
            count = float(IN * O_S + O_S)
            klt = misc.tile([1, 1], F32, tag="klt")
            nc.vector.tensor_scalar(
                klt, tall[0:1, :], count, -0.5, OP.add, OP.mult
            )
            nc.sync.dma_start(kl, klt)


_NC_CACHE = {}


def _get_nc():
    key = "full"
    if key not in _NC_CACHE:
        nc = bacc.Bacc("TRN2", target_bir_lowering=False, debug=False)
        build_bayes_kernel(
            nc, IN_FULL, B_FULL // B_SHARDS, OUT_FULL // O_SHARDS
        )
        nc.compile()
        _NC_CACHE[key] = nc
    return _NC_CACHE[key]


def _shard_inputs(x, weight_mu, weight_sigma, bias_mu, bias_sigma, eps_w, eps_b):
    B_S = B_FULL // B_SHARDS
    O_S = OUT_FULL // O_SHARDS
    f = np.float32
    xT = np.ascontiguousarray(np.asarray(x, dtype=f).T)  # [IN, B]
    weight_mu = np.asarray(weight_mu, dtype=f)
    weight_sigma = np.asarray(weight_sigma, dtype=f)
    eps_w = np.asarray(eps_w, dtype=f)
    bias_mu = np.asarray(bias_mu, dtype=f)
    bias_sigma = np.asarray(bias_sigma, dtype=f)
    eps_b = np.asarray(eps_b, dtype=f)

    wT = {}
    for o in range(O_SHARDS):
        osl = slice(o * O_S, (o + 1) * O_S)
        wT[o] = (
            np.ascontiguousarray(weight_mu[osl, :].T),
            np.ascontiguousarray(weight_sigma[osl, :].T),
            np.ascontiguousarray(eps_w[osl, :].T),
            bias_mu[osl].reshape(1, O_S).copy(),
            bias_sigma[osl].reshape(1, O_S).copy(),
            eps_b[osl].reshape(1, O_S).copy(),
        )

    in_maps = []
    for c in range(N_CORES):
        b, o = divmod(c, O_SHARDS)
        wmu_t, wsig_t, weps_t, bm, bs, be = wT[o]
        in_maps.append(
            {
                "xt": np.ascontiguousarray(xT[:, b * B_S : (b + 1) * B_S]),
                "wmu": wmu_t,
                "wsig": wsig_t,
                "weps": weps_t,
                "bmu": bm,
                "bsig": bs,
                "beps": be,
            }
        )
    return in_maps


def kernel(x, weight_mu, weight_sigma, bias_mu, bias_sigma, eps_w, eps_b, **run_kwargs):
    B_S = B_FULL // B_SHARDS
    O_S = OUT_FULL // O_SHARDS
    in_maps = _shard_inputs(
        x, weight_mu, weight_sigma, bias_mu, bias_sigma, eps_w, eps_b
    )
    nc = _get_nc()
    res = run_bass_kernel_spmd(nc, in_maps, core_ids=list(range(N_CORES)), **run_kwargs)
    out = np.empty((B_FULL, OUT_FULL), np.float32)
    for c in range(N_CORES):
        b, o = divmod(c, O_SHARDS)
        out[b * B_S : (b + 1) * B_S, o * O_S : (o + 1) * O_S] = res.results[c]["out"]
    # each (weight, bias) o-shard's KL is computed identically on both b-shard
    # rows of the mesh; take the b_shard==0 row only.
    kl_val = np.float32(sum(float(res.results[c]["kl"][0, 0]) for c in range(O_SHARDS)))
    if run_kwargs:
        kernel.last_results = res
    return out, kl_val


# revision 14
# speedup vs baseline: 1.0244x; 1.0244x over previous
"""Bayesian linear layer (reparameterized sample + KL) on 8 Trainium2 NeuronCores.

Reference computation (all fp32):
    weight = weight_mu + exp(weight_sigma) * eps_w          # [OUT, IN]
    bias   = bias_mu   + exp(bias_sigma)   * eps_b          # [OUT]
    out    = x @ weight.T + bias                            # [B, OUT]
    kl     = -0.5 * sum(1 + 2*ws - wm^2 - exp(2*ws))        # over weight
             -0.5 * sum(1 + 2*bs - bm^2 - exp(2*bs))        # over bias

Sharding: 2x4 mesh.  Batch split in 2 (B_S=4096 rows/core), out_features in 4
(O_S=1024 cols/core); core c handles b_shard=c//4, o_shard=c%4.  This gives the
lowest per-core HBM traffic (x shard 67MB + weight params 50MB + out 17MB) and
keeps the kernel PE-bound on float32r matmuls (1 col/cycle at N=512).

Host-side layout work (part of the sharding strategy):
  - x is passed pre-transposed AND pre-tiled as [BT, 128, KT, 128] so each
    batch-tile load is one DMA with 16KB-contiguous per-partition runs.
  - weight mu/sigma/eps are interleaved per k-tile as [KT, 128, 3, O_S]
    (12KB-contiguous per partition) -> one DMA per k-tile.
The contraction dim lands on SBUF partitions with no on-device transposes.

Device kernel per core:
  phase 1: per k-tile, load packed params (scalar-engine HWDGE ring), build
           W^T tile (W = mu + exp(sigma)*eps, rounded once to f32r) and
           accumulate KL partial sums on ACT/DVE.
  phase 2: per 128-row batch tile, load x tile (sync HWDGE ring), accumulate
           out[128, O_S] over 32 k-tiles into PSUM, add bias, store (SWDGE).
  tail:    fold KL partials into one scalar.
"""

import sys

import numpy as np

try:
    import concourse.bass as bass  # noqa: F401
except ImportError:  # pragma: no cover
    sys.path.insert(0, "/opt/trn_rl_repo")

import concourse.bass as bass
import concourse.tile as tile
from concourse import bacc, bass_isa, mybir

P = 128
B_FULL, IN_FULL, OUT_FULL = 8192, 4096, 4096
B_SHARDS, O_SHARDS = 2, 4
N_CORES = 8

F32 = mybir.dt.float32
MM_DT = mybir.dt.float32r  # PE fast-fp32 mode: 1 cycle/row at N>=256

AF = mybir.ActivationFunctionType
OP = mybir.AluOpType
AX = mybir.AxisListType


def build_bayes_kernel(nc, IN, B_S, O_S, mm_dt=MM_DT):
    """Emit the per-core SPMD program. Tensors are declared on nc."""
    KT = IN // P        # k tiles
    BT = B_S // P       # batch tiles
    NFREE = 512         # matmul moving free dim (1 psum bank of fp32)
    OC = O_S // NFREE   # out chunks per batch tile

    xt = nc.dram_tensor("xt", [BT, P, KT, P], mm_dt, kind="ExternalInput").ap()
    wpk = nc.dram_tensor("wpk", [KT, P, 3, O_S], F32, kind="ExternalInput").ap()
    bpk = nc.dram_tensor("bpk", [1, 3, O_S], F32, kind="ExternalInput").ap()
    out = nc.dram_tensor("out", [B_S, O_S], F32, kind="ExternalOutput").ap()
    kl = nc.dram_tensor("kl", [1, 1], F32, kind="ExternalOutput").ap()

    out_r = out.rearrange("(bt p) o -> p bt o", p=P)

    with tile.TileContext(nc) as tc:
        with (
            tc.tile_pool(name="wpool", bufs=1) as wpool,
            tc.tile_pool(name="gen", bufs=2) as gen,
            tc.tile_pool(name="xin", bufs=2) as xin,
            tc.tile_pool(name="ost", bufs=2) as ost,
            tc.tile_pool(name="misc", bufs=1) as misc,
            tc.tile_pool(name="psum", bufs=4, space="PSUM") as psum,
        ):
            # Persistent state
            w_tiles = [
                wpool.tile([P, O_S], mm_dt, tag=f"w{kt}", name=f"w{kt}")
                for kt in range(KT)
            ]
            ssig = misc.tile([P, KT], F32, tag="ssig")   # per-ktile sum(sigma)
            smu2 = misc.tile([P, KT], F32, tag="smu2")   # sum(mu^2)
            sv2 = misc.tile([P, KT], F32, tag="sv2")     # sum(exp(2 sigma))
            b_bc = misc.tile([P, O_S], F32, tag="bbc")   # bias broadcast
            klb = misc.tile([1, 4], F32, tag="klb")      # bias kl: ssig, smu2, sv2, tmp

            # Prefetch the first x tiles on the sync ring; they flow while the
            # scalar ring streams the 50MB of weight params.
            xpre = []
            for bt in range(min(2, BT)):
                xs = xin.tile([P, KT, P], mm_dt, tag="xs", name=f"xpre{bt}")
                nc.sync.dma_start(xs, xt[bt])
                xpre.append(xs)

            # ---- Phase 1: W = mu + exp(sigma)*eps, KL partial sums ----
            for kt in range(KT):
                g = gen.tile([P, 3, O_S], F32, tag="wpk")
                nc.scalar.dma_start(g, wpk[kt])
                sig, mu, eps = g[:, 0, :], g[:, 1, :], g[:, 2, :]
                wtmp = gen.tile([P, O_S], F32, tag="wtmp")
                # exp(2s) summed; the tensor output is a scratch dump
                nc.scalar.activation(
                    wtmp, sig, AF.Exp, scale=2.0, accum_out=sv2[:, kt : kt + 1]
                )
                nc.vector.tensor_reduce(ssig[:, kt : kt + 1], sig, AX.X, OP.add)
                nc.scalar.activation(wtmp, sig, AF.Exp)
                nc.vector.tensor_tensor(wtmp, wtmp, eps, OP.mult)
                # final add writes the f32r matmul operand (single rounding)
                nc.vector.tensor_tensor(w_tiles[kt], wtmp, mu, OP.add)
                # mu^2 summed; dump lands in the dead eps slot
                nc.scalar.activation(
                    eps, mu, AF.Square, accum_out=smu2[:, kt : kt + 1]
                )

            # ---- Bias: value + KL pieces ----
            # all SBUF operands of an op share start partition 0; dead slots
            # of the packed tile double as ACT dump targets.
            bt_ = gen.tile([1, 3, O_S], F32, tag="wpk", name="biastile")
            nc.sync.dma_start(bt_, bpk[0])
            bsig, bmu, beps = bt_[:, 0, :], bt_[:, 1, :], bt_[:, 2, :]
            bv = b_bc[0:1, :]
            nc.vector.tensor_reduce(klb[:, 0:1], bsig, AX.X, OP.add)
            nc.scalar.activation(bv, bsig, AF.Exp)
            nc.vector.tensor_tensor(bv, bv, beps, OP.mult)   # beps dead after
            nc.vector.tensor_tensor(bv, bv, bmu, OP.add)
            nc.scalar.activation(
                beps, bsig, AF.Exp, scale=2.0, accum_out=klb[:, 2:3]
            )
            nc.scalar.activation(bsig, bmu, AF.Square, accum_out=klb[:, 1:2])
            nc.gpsimd.partition_broadcast(b_bc, bv)

            # ---- Phase 2: out[bt] = x[bt] @ W^T + bias ----
            for bt in range(BT):
                if bt < len(xpre):
                    xs = xpre[bt]
                else:
                    xs = xin.tile([P, KT, P], mm_dt, tag="xs")
                    nc.sync.dma_start(xs, xt[bt])
                osb = ost.tile([P, O_S], F32, tag="osb")
                for oc in range(OC):
                    sl = slice(oc * NFREE, (oc + 1) * NFREE)
                    ps = psum.tile([P, NFREE], F32, tag="ps")
                    for kt in range(KT):
                        nc.tensor.matmul(
                            ps,
                            xs[:, kt, :],
                            w_tiles[kt][:, sl],
                            start=(kt == 0),
                            stop=(kt == KT - 1),
                        )
                    nc.vector.tensor_tensor(osb[:, sl], ps, b_bc[:, sl], OP.add)
                nc.sync.dma_start(out_r[:, bt, :], osb)

            # ---- KL tail ----
            rs = misc.tile([P, 1], F32, tag="rs")
            rm = misc.tile([P, 1], F32, tag="rm")
            rv = misc.tile([P, 1], F32, tag="rv")
            nc.vector.tensor_reduce(rs, ssig, AX.X, OP.add)
            nc.vector.tensor_reduce(rm, smu2, AX.X, OP.add)
            nc.vector.tensor_reduce(rv, sv2, AX.X, OP.add)
            tcol = misc.tile([P, 1], F32, tag="tcol")
            # tcol = 2*rs - rm - rv
            nc.vector.scalar_tensor_tensor(tcol, rs, 2.0, rm, OP.mult, OP.subtract)
            nc.vector.tensor_tensor(tcol, tcol, rv, OP.subtract)
            # bias terms fold into partition 0
            nc.vector.scalar_tensor_tensor(
                klb[:, 3:4], klb[:, 0:1], 2.0, klb[:, 1:2], OP.mult, OP.subtract
            )
            nc.vector.tensor_tensor(klb[:, 3:4], klb[:, 3:4], klb[:, 2:3], OP.subtract)
            nc.vector.tensor_tensor(tcol[0:1, :], tcol[0:1, :], klb[:, 3:4], OP.add)
            tall = misc.tile([P, 1], F32, tag="tall")
            nc.gpsimd.partition_all_reduce(tall, tcol, P, bass_isa.ReduceOp.add)
            # kl = -0.5 * (count + sum(2s - m^2 - v^2))
            count = float(IN * O_S + O_S)
            klt = misc.tile([1, 1], F32, tag="klt")
            nc.vector.tensor_scalar(klt, tall[0:1, :], count, -0.5, OP.add, OP.mult)
            nc.sync.dma_start(kl, klt)


_NC_CACHE = {}


def _get_nc():
    key = "full"
    if key not in _NC_CACHE:
        nc = bacc.Bacc("TRN2", target_bir_lowering=False, debug=False)
        build_bayes_kernel(nc, IN_FULL, B_FULL // B_SHARDS, OUT_FULL // O_SHARDS)
        nc.compile()
        _NC_CACHE[key] = nc
    return _NC_CACHE[key]


def _pack_x(x_shard, BT, KT):
    # [B_S, IN] -> [BT, P, KT, P] with [bt, p(=feature in tile), kt, b]
    x4 = x_shard.reshape(BT, P, KT, P)          # [bt, b, kt, p_feature]
    return np.ascontiguousarray(x4.transpose(0, 3, 2, 1))


def _pack_w(sig, mu, eps, KT, O_S):
    # each [O_S, IN] -> packed [KT, P, 3, O_S] with feature on partition
    stk = np.stack([sig.T, mu.T, eps.T], axis=1)   # [IN, 3, O_S]
    return np.ascontiguousarray(stk.reshape(KT, P, 3, O_S))


def _shard_inputs(x, weight_mu, weight_sigma, bias_mu, bias_sigma, eps_w, eps_b):
    B_S = B_FULL // B_SHARDS
    O_S = OUT_FULL // O_SHARDS
    BT, KT = B_S // P, IN_FULL // P
    f = np.float32
    x = np.asarray(x, dtype=f)
    weight_mu = np.asarray(weight_mu, dtype=f)
    weight_sigma = np.asarray(weight_sigma, dtype=f)
    eps_w = np.asarray(eps_w, dtype=f)
    bias_mu = np.asarray(bias_mu, dtype=f)
    bias_sigma = np.asarray(bias_sigma, dtype=f)
    eps_b = np.asarray(eps_b, dtype=f)

    xb = [_pack_x(x[b * B_S : (b + 1) * B_S], BT, KT) for b in range(B_SHARDS)]
    wb, bb = [], []
    for o in range(O_SHARDS):
        osl = slice(o * O_S, (o + 1) * O_S)
        wb.append(
            _pack_w(weight_sigma[osl], weight_mu[osl], eps_w[osl], KT, O_S)
        )
        bb.append(
            np.ascontiguousarray(
                np.stack([bias_sigma[osl], bias_mu[osl], eps_b[osl]])[None]
            )
        )

    in_maps = []
    for c in range(N_CORES):
        b, o = divmod(c, O_SHARDS)
        in_maps.append({"xt": xb[b], "wpk": wb[o], "bpk": bb[o]})
    return in_maps


def kernel(x, weight_mu, weight_sigma, bias_mu, bias_sigma, eps_w, eps_b, **run_kwargs):
    from concourse.bass_utils import run_bass_kernel_spmd

    B_S = B_FULL // B_SHARDS
    O_S = OUT_FULL // O_SHARDS
    in_maps = _shard_inputs(
        x, weight_mu, weight_sigma, bias_mu, bias_sigma, eps_w, eps_b
    )
    nc = _get_nc()
    res = run_bass_kernel_spmd(nc, in_maps, core_ids=list(range(N_CORES)), **run_kwargs)
    out = np.empty((B_FULL, OUT_FULL), np.float32)
    for c in range(N_CORES):
        b, o = divmod(c, O_SHARDS)
        out[b * B_S : (b + 1) * B_S, o * O_S : (o + 1) * O_S] = res.results[c]["out"]
    # each (weight, bias) o-shard's KL is identical on both mesh rows; take row 0
    kl_val = np.float32(sum(float(res.results[c]["kl"][0, 0]) for c in range(O_SHARDS)))
    if run_kwargs:
        kernel.last_results = res
    return out, kl_val
